# revision 1
# baseline (speedup 1.0000x reference)
"""CrossAttention Trainium2 kernel (8 NeuronCores).

Reference computation (B=2, N=M=2048, D=1024, H=16, C=64):
    q = rmsnorm(querys @ Wq.T, gq) * C**-0.5       [B,N,D]
    k = rmsnorm(key_feats @ Wk.T, gk)              [B,M,D]
    v = key_feats @ Wv.T                           [B,M,D]
    attn = softmax(mask(q @ k.T per head))         [B,H,N,M]
    out = (attn @ v per head, concat) @ Wo.T + bo  [B,N,D]

Sharding: core = b*4 + j (b in {0,1}; j in {0..3} owns heads 4j..4j+3 = a
256-wide slice of D). Host pre-transposes inputs/weights, folds gq*scale /
gk into Wq / Wk rows, and pre-rounds everything to f32r (fp32 with 11-bit
mantissa -> full PE rate). Per core:

  - q'^T / k'^T projections in d-slice layout [256, 2048] (contraction over
    E in the partition dim), v in [2048, 256]. q' = gs_q * q_raw etc.
  - rmsnorm sum-of-squares over the FULL D: per-core partial sumsq is
    computed by a matmul against a 1/gs^2-weighted column (compensating the
    folded gains) and AllReduced (8KB) across the 4 cores of each b; the
    collectives are emitted right after their producing phase so they hide
    behind the next projection.
  - rstd chains run lane-parallel in [128,16] layout. rstd_k is NOT applied
    to k': in the S^T = k'q'^T orientation the softmax logit scale rstd_k[m]
    is per-partition, so it folds into the exp ACTIVATE as its scale operand
    (and the mask as its bias: 0 / -1e30). rstd_q is applied to q' via a
    PE-transpose into row layout + ones outer-product broadcast.
  - attention per head: for each m-tile, 4 QK matmuls (one per 512-wide
    n-block, shared k stationary) -> batched exp -> 4 PV matmuls into a
    4-bank accumulator. v carries a 65th column of ones so row 64 of the
    accumulator is the softmax denominator (reciprocal_approx_fast + ones
    outer-product broadcast + one multiply normalizes the head output).
  - out projection produces a partial out^T [1024, 2048] (contraction over
    this core's d-slice only); the host sums 4 partials per b and adds bo.
"""

import os

import numpy as np

import concourse.tile as tile
from concourse import bacc, mybir
from concourse.bass_utils import run_bass_kernel_spmd

DEBUG = bool(os.environ.get("BASSK_DEBUG"))

B, N, M, D, H = 2, 2048, 2048, 1024, 16
C = D // H  # 64, head dim
E = D  # input feature dim
EPS = 1e-6
SCALE = C ** (-0.5)
DS = D // 4  # 256, per-core d-slice
NCORES = 8

f32 = mybir.dt.float32
f32r = mybir.dt.float32r
AF = mybir.ActivationFunctionType

NEG = -1e30


def round_f32r(x: np.ndarray) -> np.ndarray:
    b = np.ascontiguousarray(x, dtype=np.float32).view(np.uint32)
    b = (b + 0x800) & np.uint32(0xFFFFF000)
    return b.view(np.float32)


def build():
    nc = bacc.Bacc(None, target_bir_lowering=False)

    qT_d = nc.declare_dram_parameter("qT", [E, N], f32r, isOutput=False)
    kfT_d = nc.declare_dram_parameter("kfT", [E, M], f32r, isOutput=False)
    wqT_d = nc.declare_dram_parameter("wqT", [E, DS], f32r, isOutput=False)
    wkT_d = nc.declare_dram_parameter("wkT", [E, DS], f32r, isOutput=False)
    wvT_d = nc.declare_dram_parameter("wvT", [E, DS], f32r, isOutput=False)
    woT_d = nc.declare_dram_parameter("woT", [DS, D], f32r, isOutput=False)
    ig2q_d = nc.declare_dram_parameter("ig2q", [2, 128], f32r, isOutput=False)
    ig2k_d = nc.declare_dram_parameter("ig2k", [2, 128], f32r, isOutput=False)
    mb_d = nc.declare_dram_parameter("mbias", [16, 128], f32, isOutput=False)
    outT_d = nc.declare_dram_parameter("outT", [D, N], f32, isOutput=True)
    if DEBUG:
        dbg_q = nc.declare_dram_parameter("dbg_q", [128, 2, 4, 512], f32r, isOutput=True)
        dbg_k = nc.declare_dram_parameter("dbg_k", [128, 2, 4, 512], f32r, isOutput=True)
        dbg_v = nc.declare_dram_parameter("dbg_v", [128, 16, 4, C + 1], f32r, isOutput=True)
        dbg_x = nc.declare_dram_parameter("dbg_x", [128, 2, 4, 512], f32r, isOutput=True)
        dbg_rk = nc.declare_dram_parameter("dbg_rk", [128, 16], f32, isOutput=True)
        dbg_rq = nc.declare_dram_parameter("dbg_rq", [1, 2048], f32, isOutput=True)
        dbg_s = nc.declare_dram_parameter("dbg_s", [128, 2, 512], f32, isOutput=True)
        dbg_p = nc.declare_dram_parameter("dbg_p", [128, 2, 512], f32r, isOutput=True)
        dbg_o = nc.declare_dram_parameter("dbg_o", [C + 1, 2, 512], f32, isOutput=True)
        dbg_rd = nc.declare_dram_parameter("dbg_rd", [1, 512], f32, isOutput=True)
        dbg_bc = nc.declare_dram_parameter("dbg_bc", [C, 512], f32, isOutput=True)

    with (
        nc.allow_low_precision(reason="f32r matmul operands by design; fp32 PSUM"),
        tile.TileContext(nc) as tc,
    ):
        with (
            tc.tile_pool(name="singles", bufs=1) as singles,
            tc.tile_pool(name="wts", bufs=2) as wts,
            tc.tile_pool(name="blk", bufs=1 if DEBUG else 2) as blkpool,
            tc.tile_pool(name="sq", bufs=2) as sqpool,
            tc.tile_pool(name="psb", bufs=3) as ppool,
            tc.tile_pool(name="obuf", bufs=2) as obuf,
            tc.tile_pool(name="rdp", bufs=8) as rdp,
            tc.tile_pool(name="small", bufs=2) as small,
            tc.tile_pool(name="dram", bufs=1, space="DRAM") as dram,
        ):
            # ---- constants / small inputs ----
            ones_f = singles.tile([128, 64], f32)
            nc.vector.memset(ones_f, 1.0)
            ones1x64 = singles.tile([1, 64], f32)
            nc.vector.memset(ones1x64, 1.0)
            ones1x128 = singles.tile([1, 128], f32)
            nc.vector.memset(ones1x128, 1.0)
            eps_t = singles.tile([128, 1], f32)
            nc.vector.memset(eps_t, EPS)
            invd_t = singles.tile([128, 1], f32)
            nc.vector.memset(invd_t, 1.0 / D)
            ig2q_sb = singles.tile([128, 2], f32r)
            nc.sync.dma_start(out=ig2q_sb, in_=ig2q_d.rearrange("t p -> p t"))
            ig2k_sb = singles.tile([128, 2], f32r)
            nc.sync.dma_start(out=ig2k_sb, in_=ig2k_d.rearrange("t p -> p t"))
            mb_sb = singles.tile([128, 16], f32)
            nc.sync.dma_start(out=mb_sb, in_=mb_d.rearrange("t p -> p t"))

            # weights rotate through 2 pool slots: wq,wk up front; wv,wo reuse
            wq_sb = wts.tile([128, 8, DS], f32r, tag="w")
            wk_sb = wts.tile([128, 8, DS], f32r, tag="w")
            for et in range(8):
                nc.sync.dma_start(out=wq_sb[:, et, :], in_=wqT_d[et * 128 : et * 128 + 128, :])
                nc.scalar.dma_start(out=wk_sb[:, et, :], in_=wkT_d[et * 128 : et * 128 + 128, :])

            # ---- persistent activations ----
            qT = singles.tile([128, 2, 4, 512], f32r)  # [p, dt, nb, n]
            kT = singles.tile([128, 2, 4, 512], f32r)  # [p, dt, mb, m]
            v_sb = singles.tile([128, 16, 4, C + 1], f32r)  # [m_p, mt, h, c|ones]
            xT = singles.tile([128, 2, 4, 512], f32r)  # [p, dt, nb, n]
            nc.vector.tensor_copy(
                v_sb[:, :, :, C], ones_f.rearrange("p (a b) -> p a b", a=16)
            )

            ccq_in = dram.tile([2048], f32)
            ccq_out = dram.tile([2048], f32)
            cck_in = dram.tile([2048], f32)
            cck_out = dram.tile([2048], f32)

            def projection(src_d, w_sb, dst, ig2_sb, cc_in_t, dma_eng):
                """dst[dt, nb] = W'^T-slice @ src-block; partial sumsq -> cc_in."""
                for nb in range(4):
                    blk = blkpool.tile([128, 8, 512], f32r, tag="blk")
                    for et in range(8):
                        dma_eng.dma_start(
                            out=blk[:, et, :],
                            in_=src_d[et * 128 : et * 128 + 128, nb * 512 : nb * 512 + 512],
                        )
                    ss_ps = ssps.tile([1, 512], f32, tag="ss")
                    for dt in range(2):
                        ps = projps.tile([128, 512], f32, tag="proj")
                        for et in range(8):
                            nc.tensor.matmul(
                                ps,
                                w_sb[:, et, dt * 128 : dt * 128 + 128],
                                blk[:, et, :],
                                start=(et == 0),
                                stop=(et == 7),
                            )
                        nc.vector.tensor_copy(dst[:, dt, nb, :], ps)
                        sq = sqpool.tile([128, 512], f32r, tag="sq")
                        nc.vector.tensor_mul(sq, dst[:, dt, nb, :], dst[:, dt, nb, :])
                        nc.tensor.matmul(
                            ss_ps,
                            ig2_sb[:, dt : dt + 1],
                            sq,
                            start=(dt == 0),
                            stop=(dt == 1),
                            skip_group_check=True,
                        )
                    ss_sb = small.tile([1, 512], f32, tag="ss_sb")
                    nc.scalar.copy(ss_sb, ss_ps)
                    nc.sync.dma_start(
                        out=cc_in_t[nb * 512 : nb * 512 + 512].rearrange(
                            "(a n) -> a n", a=1
                        ),
                        in_=ss_sb,
                    )

            def rstd128(cc_out_t, tag):
                """[128,16] lane-parallel rstd chain: p,t -> 1/sqrt(ss/D+eps)."""
                ss128 = small.tile([128, 16], f32, tag=f"ss128{tag}")
                nc.sync.dma_start(
                    out=ss128, in_=cc_out_t.rearrange("(t p) -> p t", p=128)
                )
                std = small.tile([128, 16], f32, tag=f"std{tag}")
                nc.scalar.activation(std, ss128, AF.Sqrt, bias=eps_t, scale=invd_t)
                r = singles.tile([128, 16], f32)
                nc.vector.reciprocal_approx_fast(out=r, in_=std)
                return r

            with (
                tc.tile_pool(name="projps", bufs=2, space="PSUM") as projps,
                tc.tile_pool(name="vps", bufs=2, space="PSUM") as vps,
                tc.tile_pool(name="ssps", bufs=2, space="PSUM") as ssps,
            ):
                # ---- q projection, then its collective (hidden behind k/v) ----
                projection(qT_d, wq_sb, qT, ig2q_sb, ccq_in, nc.sync)
                nc.gpsimd.collective_compute(
                    "AllReduce",
                    mybir.AluOpType.add,
                    replica_groups=[[0, 1, 2, 3], [4, 5, 6, 7]],
                    ins=[ccq_in.opt()],
                    outs=[ccq_out.opt()],
                )

                # ---- k projection, then its collective (hidden behind v) ----
                projection(kfT_d, wk_sb, kT, ig2k_sb, cck_in, nc.scalar)
                nc.gpsimd.collective_compute(
                    "AllReduce",
                    mybir.AluOpType.add,
                    replica_groups=[[0, 1, 2, 3], [4, 5, 6, 7]],
                    ins=[cck_in.opt()],
                    outs=[cck_out.opt()],
                )

                # ---- v projection (kfT re-streamed) ----
                wv_sb = wts.tile([128, 8, DS], f32r, tag="w")
                for et in range(8):
                    nc.scalar.dma_start(out=wv_sb[:, et, :], in_=wvT_d[et * 128 : et * 128 + 128, :])
                for mb in range(4):
                    blk = blkpool.tile([128, 8, 512], f32r, tag="blk")
                    for et in range(8):
                        nc.scalar.dma_start(
                            out=blk[:, et, :],
                            in_=kfT_d[et * 128 : et * 128 + 128, mb * 512 : mb * 512 + 512],
                        )
                    for mt in range(4):
                        psv = vps.tile([128, 256], f32, tag="v")
                        for et in range(8):
                            nc.tensor.matmul(
                                psv,
                                blk[:, et, mt * 128 : mt * 128 + 128],
                                wv_sb[:, et, :],
                                start=(et == 0),
                                stop=(et == 7),
                            )
                        nc.vector.tensor_copy(
                            v_sb[:, mb * 4 + mt, :, 0:C],
                            psv.rearrange("p (h c) -> p h c", c=C),
                        )

                # ---- rstd_k: [128,16] lane-parallel; feeds exp scale directly ----
                rstdk = rstd128(cck_out, "k")

                # ---- rstd_q: row layout [1, 2048] for the bcast outer-products ----
                ssq_row = singles.tile([1, 2048], f32)
                nc.sync.dma_start(
                    out=ssq_row, in_=ccq_out.rearrange("(a n) -> a n", a=1)
                )
                nc.scalar.activation(
                    ssq_row, ssq_row, AF.Sqrt, bias=eps_t[0:1, :], scale=invd_t[0:1, :]
                )
                rs_row = singles.tile([1, 2048], f32)
                nc.vector.reciprocal_approx_fast(out=rs_row, in_=ssq_row)
                # q finalize: qT[d, n] *= rstd_q[n] via ones outer-product bcast
                for nb in range(4):
                    bcq = projps.tile([128, 512], f32, tag="proj")
                    nc.tensor.matmul(
                        bcq,
                        ones1x128,
                        rs_row[:, nb * 512 : nb * 512 + 512],
                        start=True,
                        stop=True,
                    )
                    for dt in range(2):
                        nc.vector.tensor_mul(qT[:, dt, nb, :], qT[:, dt, nb, :], bcq)

                # ---- HAM warm-up burst: ~5us of dense dependency-free matmuls
                # (the collective/norm stall re-throttles the PE clock to 4/8;
                # a fully-busy 3.4us window is needed to flip it back to 8/8
                # before the attention stream, whose fine-grained gaps can
                # never re-warm it)
                warm = projps.tile([128, 512], f32, tag="proj")
                for i in range(20):
                    nc.tensor.matmul(
                        warm,
                        kT[:, 0, 0, 0:128],
                        kT[:, 0, 1, :],
                        start=(i == 0),
                        stop=(i == 19),
                        skip_group_check=True,
                    )
                warm_sink = small.tile([1, 512], f32, tag="rd")
                nc.vector.tensor_copy(warm_sink, warm[0:1, :])

            # ---- phase 3: attention over (head, nb-pair) passes ----
            # PSUM: s2 (2 banks x 2 bufs) + o2 (2 banks) + dummy (1) = 7 banks.
            # A dependency-free dummy matmul per m-tile bridges the PE's
            # ~150ns/mt deficit vs the ACT exp pacing: any recurring PE gap
            # keeps the HAM clock-gate at K=4/8 (half clock), which costs far
            # more than the dummy's 213ns.
            with (
                tc.tile_pool(name="sps", bufs=2, space="PSUM") as spool,
                tc.tile_pool(name="ops", bufs=1, space="PSUM") as opool,
                tc.tile_pool(name="dmy", bufs=1, space="PSUM") as dmypool,
            ):
                dum = dmypool.tile([128, 512], f32, tag="dum")

                def emit_dummy():
                    nc.tensor.matmul(
                        dum, kT[:, 0, 0, 0:128], kT[:, 0, 1, :],
                        start=True, stop=True, skip_group_check=True,
                    )

                def emit_normalize(state):
                    """bc outer-products + muls for a pass whose DVE recips are
                    done by now (emitted one pass late to keep PE gapless)."""
                    hh, nbp, oo_sb, rds = state
                    ddt, ooff = hh // 2, (hh % 2) * C
                    for i, nb in enumerate((2 * nbp, 2 * nbp + 1)):
                        bc = spool.tile([128, 2, 512], f32, tag="s2")
                        nc.tensor.matmul(
                            bc[0:C, 0, :], ones1x64, rds[i], start=True, stop=True
                        )
                        nc.vector.tensor_mul(
                            xT[ooff : ooff + C, ddt, nb, :],
                            oo_sb[0:C, i, :],
                            bc[0:C, 0, :],
                        )

                prev = None
                for h in range(4):
                    dt, off = h // 2, (h % 2) * C
                    for nbp in range(2):
                        nbs = (2 * nbp, 2 * nbp + 1)
                        o2 = opool.tile([C + 1, 2, 512], f32, tag="o2")
                        for mt in range(16):
                            kT_lhs = kT[
                                off : off + C, dt, mt // 4,
                                (mt % 4) * 128 : (mt % 4) * 128 + 128,
                            ]
                            s2 = spool.tile([128, 2, 512], f32, tag="s2")
                            for i, nb in enumerate(nbs):
                                nc.tensor.matmul(
                                    s2[:, i, :],
                                    kT_lhs,
                                    qT[off : off + C, dt, nb, :],
                                    start=True,
                                    stop=True,
                                )
                            p2 = ppool.tile([128, 2, 512], f32r, tag="p")
                            nc.scalar.activation(
                                p2, s2, AF.Exp,
                                bias=mb_sb[:, mt : mt + 1],
                                scale=rstdk[:, mt : mt + 1],
                            )
                            if DEBUG and h == 0 and mt == 0 and nbp == 0:
                                s_sb = ppool.tile([128, 2, 512], f32, tag="dbgs")
                                nc.vector.tensor_copy(s_sb, s2)
                                nc.sync.dma_start(out=dbg_s[:], in_=s_sb)
                                nc.sync.dma_start(out=dbg_p[:], in_=p2)
                            for i in range(2):
                                nc.tensor.matmul(
                                    o2[:, i, :],
                                    v_sb[:, mt, h, :],
                                    p2[:, i, :],
                                    start=(mt == 0),
                                    stop=(mt == 15),
                                    skip_group_check=True,
                                )
                            emit_dummy()
                        # free o2 with a single copy; recips run during the
                        # next pass, bc+mul are emitted one pass late
                        o_sb = obuf.tile([C + 1, 2, 512], f32, tag="osb")
                        nc.vector.tensor_copy(o_sb, o2)
                        if DEBUG and h == 0 and nbp == 0:
                            nc.sync.dma_start(out=dbg_o[:], in_=o_sb)
                        rds = []
                        for i in range(2):
                            den_sb = rdp.tile([1, 512], f32, tag="den")
                            nc.vector.tensor_copy(den_sb, o_sb[C : C + 1, i, :])
                            rd = rdp.tile([1, 512], f32, tag="rd")
                            nc.vector.reciprocal_approx_fast(out=rd, in_=den_sb)
                            rds.append(rd)
                        if prev is not None:
                            emit_normalize(prev)
                        prev = (h, nbp, o_sb, rds)
                emit_normalize(prev)
                warm_sink2 = small.tile([1, 512], f32, tag="ss_sb")
                nc.vector.tensor_copy(warm_sink2, dum[0:1, :])

            if DEBUG:
                nc.sync.dma_start(out=dbg_q[:], in_=qT)
                nc.sync.dma_start(out=dbg_k[:], in_=kT)
                nc.sync.dma_start(out=dbg_v[:], in_=v_sb)
                nc.sync.dma_start(out=dbg_x[:], in_=xT)
                nc.sync.dma_start(out=dbg_rk[:], in_=rstdk)
                nc.sync.dma_start(out=dbg_rq[:], in_=rs_row)

            # ---- phase 4: out projection (partial over d-slice) ----
            with tc.tile_pool(name="outps", bufs=3, space="PSUM") as outps:
                wo_sb = wts.tile([128, 2, D], f32r, tag="w")
                for dc in range(2):
                    nc.sync.dma_start(
                        out=wo_sb[:, dc, :], in_=woT_d[dc * 128 : dc * 128 + 128, :]
                    )
                for nb in range(4):
                    for ot in range(8):
                        ps = outps.tile([128, 512], f32, tag="out")
                        for dc in range(2):
                            nc.tensor.matmul(
                                ps,
                                wo_sb[:, dc, ot * 128 : ot * 128 + 128],
                                xT[:, dc, nb, :],
                                start=(dc == 0),
                                stop=(dc == 1),
                            )
                        out_sb = ppool.tile([128, 512], f32, tag="osb")
                        nc.scalar.copy(out_sb, ps)
                        nc.sync.dma_start(
                            out=outT_d[ot * 128 : ot * 128 + 128, nb * 512 : nb * 512 + 512],
                            in_=out_sb,
                        )

    nc.finalize()
    return nc


_NC_CACHE = None


def _get_nc():
    global _NC_CACHE
    if _NC_CACHE is None:
        _NC_CACHE = build()
    return _NC_CACHE


def make_in_maps(querys, key_feats, mask, Wq, Wk, Wv, gq, gk, Wo, bo):
    querys = np.asarray(querys, dtype=np.float32)
    key_feats = np.asarray(key_feats, dtype=np.float32)
    mask = np.asarray(mask)
    gq = np.asarray(gq, dtype=np.float32)
    gk = np.asarray(gk, dtype=np.float32)

    gsq_full = gq * np.float32(SCALE)  # folded into Wq rows
    gsk_full = gk.astype(np.float32)  # folded into Wk rows
    Wq_f = np.asarray(Wq, dtype=np.float32) * gsq_full[:, None]
    Wk_f = np.asarray(Wk, dtype=np.float32) * gsk_full[:, None]

    qT = [round_f32r(querys[b].T) for b in range(B)]
    kfT = [round_f32r(key_feats[b].T) for b in range(B)]
    mb = [
        np.where(mask[b] == 0, np.float32(NEG), np.float32(0.0))
        .astype(np.float32)
        .reshape(16, 128)
        for b in range(B)
    ]
    wqT, wkT, wvT, woT, ig2q, ig2k = [], [], [], [], [], []
    for j in range(4):
        dsl = slice(j * DS, (j + 1) * DS)
        wqT.append(round_f32r(Wq_f[dsl].T))
        wkT.append(round_f32r(Wk_f[dsl].T))
        wvT.append(round_f32r(np.asarray(Wv)[dsl].T))
        woT.append(round_f32r(np.asarray(Wo)[:, dsl].T))
        # sumsq compensation: raw sumsq = sum_d (q'_d)^2 / gs_d^2
        ig2q.append(round_f32r((1.0 / gsq_full[dsl] ** 2).reshape(2, 128)))
        ig2k.append(round_f32r((1.0 / gsk_full[dsl] ** 2).reshape(2, 128)))

    in_maps = []
    for cid in range(NCORES):
        b, j = cid // 4, cid % 4
        in_maps.append(
            {
                "qT": qT[b],
                "kfT": kfT[b],
                "wqT": wqT[j],
                "wkT": wkT[j],
                "wvT": wvT[j],
                "woT": woT[j],
                "ig2q": ig2q[j],
                "ig2k": ig2k[j],
                "mbias": mb[b],
            }
        )
    return in_maps


def assemble(results, bo):
    bo = np.asarray(bo, dtype=np.float32)
    out = np.zeros((B, N, D), dtype=np.float32)
    for cid in range(NCORES):
        b = cid // 4
        out[b] += results[cid]["outT"].T
    out += bo
    return out


def kernel(querys, key_feats, mask, Wq, Wk, Wv, gq, gk, Wo, bo):
    nc = _get_nc()
    in_maps = make_in_maps(querys, key_feats, mask, Wq, Wk, Wv, gq, gk, Wo, bo)
    res = run_bass_kernel_spmd(nc, in_maps, list(range(NCORES)))
    return assemble(res.results, bo)



# revision 10
# speedup vs baseline: 1.6578x; 1.6578x over previous
"""CrossAttention Trainium2 kernel (8 NeuronCores), v2.

Reference computation (B=2, N=M=2048, D=1024, H=16, C=64):
    q = rmsnorm(querys @ Wq.T, gq) * C**-0.5       [B,N,D]
    k = rmsnorm(key_feats @ Wk.T, gk)              [B,M,D]
    v = key_feats @ Wv.T                           [B,M,D]
    attn = softmax(mask(q @ k.T per head))         [B,H,N,M]
    out = (attn @ v per head, concat) @ Wo.T + bo  [B,N,D]

Sharding: core = b*4 + j (b in {0,1}; j owns heads 4j..4j+3 = a 256-wide
d-slice). v2 changes vs v1:

  - Mask compaction: the host packs only mask==1 key rows (plus zero pad
    to Mp, a multiple of 128). Attention + k/v projections shrink ~2x.
    Pad rows are killed by the -1e30 exp bias.
  - No collectives. The full-D rmsnorm sum-of-squares is computed locally
    per core via a host-side Cholesky factor: sumsq(x) = ||L^T x||^2 with
    G = W_raw^T W_raw = L L^T. L is block-lower-triangular, so only 36 of
    64 [128,128] blocks contribute. ss = colsum(z^2) via DVE square + PE
    ones-column matmul. This removes the AllReduce bootstrap (~110us) +
    latency (~70us) and the PE idle window that collapsed the HAM PE
    clock to 4/8 for the whole attention phase.
  - kfT streamed once: k-proj, z_k and v-proj all consume the same SBUF
    block. Input DMA spread over sync/gpsimd/scalar queues.
  - Out-projection interleaved into the attention instruction stream
    (one ot-chunk per m-tile) once an n-block's xT is finalized; only the
    last two n-blocks drain after attention.
"""

import numpy as np

import concourse.tile as tile
from concourse import bacc, mybir
from concourse.bass_utils import run_bass_kernel_spmd

B, N, M, D, H = 2, 2048, 2048, 1024, 16
C = D // H  # 64, head dim
E = D  # input feature dim
EPS = 1e-6
SCALE = C ** (-0.5)
DS = D // 4  # 256, per-core d-slice
NCORES = 8

f32 = mybir.dt.float32
f32r = mybir.dt.float32r
AF = mybir.ActivationFunctionType

NEG = -1e30

# block-lower-triangle of L in [e, zd] 128-blocks: L[e, zd] != 0 for zd <= e
TRI = [(dz, et) for dz in range(8) for et in range(dz, 8)]
NTRI = len(TRI)  # 36


def round_f32r(x: np.ndarray) -> np.ndarray:
    b = np.ascontiguousarray(x, dtype=np.float32).view(np.uint32)
    b = (b + 0x800) & np.uint32(0xFFFFF000)
    return b.view(np.float32)


def build(Mp: int):
    MT = Mp // 128
    # kf stream blocks of up to 512 columns
    W_LIST = [(s, min(512, Mp - s)) for s in range(0, Mp, 512)]

    nc = bacc.Bacc(None, target_bir_lowering=False)

    qT_d = nc.declare_dram_parameter("qT", [E, N], f32r, isOutput=False)
    kfT_d = nc.declare_dram_parameter("kfT", [E, Mp], f32r, isOutput=False)
    wqT_d = nc.declare_dram_parameter("wqT", [E, DS], f32r, isOutput=False)
    wkT_d = nc.declare_dram_parameter("wkT", [E, DS], f32r, isOutput=False)
    wvT_d = nc.declare_dram_parameter("wvT", [E, DS], f32r, isOutput=False)
    woT_d = nc.declare_dram_parameter("woT", [DS, D], f32r, isOutput=False)
    lq_d = nc.declare_dram_parameter("Lq", [128, NTRI * 128], f32r, isOutput=False)
    lk_d = nc.declare_dram_parameter("Lk", [128, NTRI * 128], f32r, isOutput=False)
    mb_d = nc.declare_dram_parameter("mbias", [MT, 128], f32, isOutput=False)
    outT_d = nc.declare_dram_parameter("outT", [D, N], f32, isOutput=True)

    with (
        nc.allow_low_precision(reason="f32r matmul operands by design; fp32 PSUM"),
        tile.TileContext(nc) as tc,
    ):
        with (
            tc.tile_pool(name="singles", bufs=1) as singles,
            tc.tile_pool(name="wts", bufs=3) as wts,
            tc.tile_pool(name="lw", bufs=2) as lpool,
            tc.tile_pool(name="blk", bufs=3) as blkpool,
            tc.tile_pool(name="sq", bufs=2) as sqpool,
            tc.tile_pool(name="psb", bufs=3) as ppool,
            tc.tile_pool(name="obuf", bufs=2) as obuf,
            tc.tile_pool(name="osb", bufs=2) as osbp,
            tc.tile_pool(name="rdp", bufs=4) as rdp,
            tc.tile_pool(name="small", bufs=4) as small,
            tc.tile_pool(name="dram", bufs=1, space="DRAM") as dram,
        ):
            # round-robin input-stream DMA queues (PE untouched; DVE busy)
            inq = [nc.sync, nc.gpsimd, nc.scalar]
            qn = [0]

            def dq():
                e = inq[qn[0] % 3]
                qn[0] += 1
                return e

            outq_eng = [nc.sync, nc.scalar]

            # ---- constants / small inputs ----
            ones_f = singles.tile([128, 64], f32)
            nc.vector.memset(ones_f, 1.0)
            ones1x64 = singles.tile([1, 64], f32)
            nc.vector.memset(ones1x64, 1.0)
            ones1x128 = singles.tile([1, 128], f32)
            nc.vector.memset(ones1x128, 1.0)
            ones_col = singles.tile([128, 1], f32r)
            nc.vector.tensor_copy(ones_col, ones_f[:, 0:1])
            eps_t = singles.tile([128, 1], f32)
            nc.vector.memset(eps_t, EPS)
            invd_t = singles.tile([128, 1], f32)
            nc.vector.memset(invd_t, 1.0 / D)
            mb_sb = singles.tile([128, MT], f32)
            nc.gpsimd.dma_start(out=mb_sb, in_=mb_d.rearrange("t p -> p t"))

            # ---- persistent activations ----
            qT = singles.tile([128, 2, 4, 512], f32r)  # [p, dt, nb, n]
            kT = singles.tile([128, 2, MT, 128], f32r)  # [p, dt, mt, m]
            v_sb = singles.tile([128, MT, 4, C + 1], f32r)  # [m_p, mt, h, c|ones]
            xT = qT  # aliased: each [h, nb] slice is written only after its last QK read
            nc.vector.tensor_copy(
                v_sb[:, :, :, C],
                ones_f[:, 0:MT * 4].rearrange("p (a b) -> p a b", a=MT),
            )
            ssq_row = singles.tile([1, 2048], f32)
            ssk_row = singles.tile([1, Mp], f32)
            ssk_d = dram.tile([Mp], f32)

            # ---- weights (consumption order) ----
            wq_sb = wts.tile([128, 8, DS], f32r, tag="w")
            for et in range(8):
                dq().dma_start(out=wq_sb[:, et, :], in_=wqT_d[et * 128 : et * 128 + 128, :])
            lq_sb = lpool.tile([128, NTRI, 128], f32r, tag="L")
            for c in range(4):
                dq().dma_start(
                    out=lq_sb[:, c * 9 : c * 9 + 9, :],
                    in_=lq_d[:, c * 9 * 128 : (c * 9 + 9) * 128].rearrange(
                        "p (a b) -> p a b", a=9
                    ),
                )
            LIDX = {b: i for i, b in enumerate(TRI)}

            with (
                tc.tile_pool(name="projps", bufs=2, space="PSUM") as projps,
                tc.tile_pool(name="zps", bufs=2, space="PSUM") as zps,
                tc.tile_pool(name="ssps", bufs=2, space="PSUM") as ssps,
                tc.tile_pool(name="vps", bufs=2, space="PSUM") as vps,
            ):
                def emit_z(blk, l_sb, ss_seg, w):
                    """ss_seg[1, w] += colsum over full zd of (L^T x)^2."""
                    ss_ps = ssps.tile([1, 512], f32, tag="ss")
                    pend = []

                    def colsum(dz, zp):
                        sq = sqpool.tile([128, 512], f32r, tag="sq")
                        nc.scalar.activation(sq[:, 0:w], zp[:, 0:w], AF.Square)
                        nc.tensor.matmul(
                            ss_ps[:, 0:w],
                            ones_col,
                            sq[:, 0:w],
                            start=(dz == 0),
                            stop=(dz == 7),
                            skip_group_check=True,
                        )

                    for dz in range(8):
                        zp = zps.tile([128, 512], f32, tag="z")
                        for et in range(dz, 8):
                            nc.tensor.matmul(
                                zp[:, 0:w],
                                l_sb[:, LIDX[(dz, et)], :],
                                blk[:, et, 0:w],
                                start=(et == dz),
                                stop=(et == 7),
                            )
                        pend.append((dz, zp))
                        if len(pend) > 1:
                            colsum(*pend.pop(0))
                    colsum(*pend.pop(0))
                    nc.vector.tensor_copy(ss_seg, ss_ps[:, 0:w])

                # ---- P1: q projection + z_q, z staggered one block behind ----
                pend_z = []
                for nb in range(4):
                    blk = blkpool.tile([128, 8, 512], f32r, tag="blk")
                    for et in range(8):
                        dq().dma_start(
                            out=blk[:, et, :],
                            in_=qT_d[et * 128 : et * 128 + 128, nb * 512 : nb * 512 + 512],
                        )
                    for dt in range(2):
                        ps = projps.tile([128, 512], f32, tag="proj")
                        for et in range(8):
                            nc.tensor.matmul(
                                ps,
                                wq_sb[:, et, dt * 128 : dt * 128 + 128],
                                blk[:, et, :],
                                start=(et == 0),
                                stop=(et == 7),
                            )
                        nc.vector.tensor_copy(qT[:, dt, nb, :], ps)
                    pend_z.append((blk, nb))
                    if nb == 1:
                        # k weights + first kf block next in queue order
                        wk_sb = wts.tile([128, 8, DS], f32r, tag="w")
                        for et in range(8):
                            dq().dma_start(
                                out=wk_sb[:, et, :],
                                in_=wkT_d[et * 128 : et * 128 + 128, :],
                            )
                    if len(pend_z) > 1:
                        b0, n0 = pend_z.pop(0)
                        emit_z(b0, lq_sb, ssq_row[:, n0 * 512 : n0 * 512 + 512], 512)
                while pend_z:
                    b0, n0 = pend_z.pop(0)
                    emit_z(b0, lq_sb, ssq_row[:, n0 * 512 : n0 * 512 + 512], 512)

                # ---- rstd_q + qT finalize ----
                nc.scalar.activation(
                    ssq_row, ssq_row, AF.Sqrt, bias=eps_t[0:1, :], scale=invd_t[0:1, :]
                )
                rs_row = singles.tile([1, 2048], f32)
                nc.vector.reciprocal_approx_fast(out=rs_row, in_=ssq_row)
                for nb in range(4):
                    bcq = projps.tile([128, 512], f32, tag="proj")
                    nc.tensor.matmul(
                        bcq,
                        ones1x128,
                        rs_row[:, nb * 512 : nb * 512 + 512],
                        start=True,
                        stop=True,
                    )
                    for dt in range(2):
                        nc.vector.tensor_mul(qT[:, dt, nb, :], qT[:, dt, nb, :], bcq)

                # ---- P2: k proj + z_k + v proj, one kf stream ----
                lk_sb = lpool.tile([128, NTRI, 128], f32r, tag="L")
                wv_sb = wts.tile([128, 8, DS], f32r, tag="w")
                for bi, (s0, w) in enumerate(W_LIST):
                    blk = blkpool.tile([128, 8, 512], f32r, tag="blk")
                    for et in range(8):
                        dq().dma_start(
                            out=blk[:, et, 0:w],
                            in_=kfT_d[et * 128 : et * 128 + 128, s0 : s0 + w],
                        )
                    if bi == 0:
                        for c in range(4):
                            dq().dma_start(
                                out=lk_sb[:, c * 9 : c * 9 + 9, :],
                                in_=lk_d[:, c * 9 * 128 : (c * 9 + 9) * 128].rearrange(
                                    "p (a b) -> p a b", a=9
                                ),
                            )
                        for et in range(8):
                            dq().dma_start(
                                out=wv_sb[:, et, :],
                                in_=wvT_d[et * 128 : et * 128 + 128, :],
                            )
                    nmt = w // 128
                    for dt in range(2):
                        ps = projps.tile([128, 512], f32, tag="proj")
                        for et in range(8):
                            nc.tensor.matmul(
                                ps[:, 0:w],
                                wk_sb[:, et, dt * 128 : dt * 128 + 128],
                                blk[:, et, 0:w],
                                start=(et == 0),
                                stop=(et == 7),
                            )
                        nc.vector.tensor_copy(
                            kT[:, dt, bi * 4 : bi * 4 + nmt, :],
                            ps[:, 0:w].rearrange("p (a b) -> p a b", a=nmt),
                        )
                    emit_z(blk, lk_sb, ssk_row[:, s0 : s0 + w], w)
                    for mtL in range(nmt):
                        psv = vps.tile([128, 256], f32, tag="v")
                        for et in range(8):
                            nc.tensor.matmul(
                                psv,
                                blk[:, et, mtL * 128 : mtL * 128 + 128],
                                wv_sb[:, et, :],
                                start=(et == 0),
                                stop=(et == 7),
                            )
                        nc.vector.tensor_copy(
                            v_sb[:, bi * 4 + mtL, :, 0:C],
                            psv.rearrange("p (h c) -> p h c", c=C),
                        )

                # ---- rstd_k: bounce [1,Mp] -> [128,MT], then sqrt+recip ----
                nc.sync.dma_start(
                    out=ssk_d.rearrange("(a n) -> a n", a=1), in_=ssk_row
                )
                ssk128 = small.tile([128, MT], f32, tag="ssk")
                nc.sync.dma_start(
                    out=ssk128, in_=ssk_d.rearrange("(t p) -> p t", p=128)
                )
                std = small.tile([128, MT], f32, tag="std")
                nc.scalar.activation(std, ssk128, AF.Sqrt, bias=eps_t, scale=invd_t)
                rstdk = singles.tile([128, MT], f32)
                nc.vector.reciprocal_approx_fast(out=rstdk, in_=std)

                # wo load (consumed mid-attention)
                wo_sb = wts.tile([128, 2, D], f32r, tag="w")
                for dc in range(2):
                    dq().dma_start(
                        out=wo_sb[:, dc, :], in_=woT_d[dc * 128 : dc * 128 + 128, :]
                    )

            # ---- P4: attention, nbp-outer, out-proj interleaved ----
            with (
                tc.tile_pool(name="sps", bufs=2, space="PSUM") as spool,
                tc.tile_pool(name="ops", bufs=1, space="PSUM") as opool,
                tc.tile_pool(name="outps", bufs=2, space="PSUM") as outps,
            ):
                def emit_outproj_chunk(nb, ot):
                    ps = outps.tile([128, 512], f32, tag="out")
                    for dc in range(2):
                        nc.tensor.matmul(
                            ps,
                            wo_sb[:, dc, ot * 128 : ot * 128 + 128],
                            xT[:, dc, nb, :],
                            start=(dc == 0),
                            stop=(dc == 1),
                            skip_group_check=True,
                        )
                    out_sb = osbp.tile([128, 512], f32, tag="osb")
                    nc.vector.tensor_copy(out_sb, ps)
                    outq_eng[(nb + ot) % 2].dma_start(
                        out=outT_d[ot * 128 : ot * 128 + 128, nb * 512 : nb * 512 + 512],
                        in_=out_sb,
                    )

                def emit_normalize(state):
                    """bc outer-products + muls for a pass whose DVE recips are
                    done by now (emitted one pass late to keep PE gapless)."""
                    hh, nbp, oo_sb, rds = state
                    ddt, ooff = hh // 2, (hh % 2) * C
                    for i, nb in enumerate((2 * nbp, 2 * nbp + 1)):
                        bc = spool.tile([128, 2, 512], f32, tag="s2")
                        nc.tensor.matmul(
                            bc[0:C, 0, :], ones1x64, rds[i], start=True, stop=True
                        )
                        nc.vector.tensor_mul(
                            xT[ooff : ooff + C, ddt, nb, :],
                            oo_sb[0:C, i, :],
                            bc[0:C, 0, :],
                        )

                prev = None
                out_chunks = []  # deferred (nb, ot) out-proj chunks
                for nbp in range(2):
                    for h in range(4):
                        dt, off = h // 2, (h % 2) * C
                        nbs = (2 * nbp, 2 * nbp + 1)
                        o2 = opool.tile([C + 1, 2, 512], f32, tag="o2")
                        for mt in range(MT):
                            kT_lhs = kT[off : off + C, dt, mt, :]
                            s2 = spool.tile([128, 2, 512], f32, tag="s2")
                            for i, nb in enumerate(nbs):
                                nc.tensor.matmul(
                                    s2[:, i, :],
                                    kT_lhs,
                                    qT[off : off + C, dt, nb, :],
                                    start=True,
                                    stop=True,
                                )
                            p2 = ppool.tile([128, 2, 512], f32r, tag="p")
                            nc.scalar.activation(
                                p2, s2, AF.Exp,
                                bias=mb_sb[:, mt : mt + 1],
                                scale=rstdk[:, mt : mt + 1],
                            )
                            for i in range(2):
                                nc.tensor.matmul(
                                    o2[:, i, :],
                                    v_sb[:, mt, h, :],
                                    p2[:, i, :],
                                    start=(mt == 0),
                                    stop=(mt == MT - 1),
                                    skip_group_check=True,
                                )
                            if out_chunks:
                                emit_outproj_chunk(*out_chunks.pop(0))
                        o_sb = obuf.tile([C + 1, 2, 512], f32, tag="osb")
                        nc.vector.tensor_copy(o_sb, o2)
                        rds = []
                        for i in range(2):
                            den_sb = rdp.tile([1, 512], f32, tag="den")
                            nc.vector.tensor_copy(den_sb, o_sb[C : C + 1, i, :])
                            rd = rdp.tile([1, 512], f32, tag="rd")
                            nc.vector.reciprocal_approx_fast(out=rd, in_=den_sb)
                            rds.append(rd)
                        if prev is not None:
                            emit_normalize(prev)
                        prev = (h, nbp, o_sb, rds)
                        if nbp == 1 and h == 0:
                            # nb0/nb1 xT finalized by normalize((0,3)) above
                            out_chunks = [(nb, ot) for nb in (0, 1) for ot in range(8)]
                emit_normalize(prev)
                for nb in (2, 3):
                    for ot in range(8):
                        emit_outproj_chunk(nb, ot)

    nc.finalize()
    return nc


_NC_CACHE = {}


def _get_nc(Mp=1024):
    if Mp not in _NC_CACHE:
        _NC_CACHE[Mp] = build(Mp)
    return _NC_CACHE[Mp]


def _chol_factor(W):
    G = W.astype(np.float64).T @ W.astype(np.float64)
    G += np.eye(E) * (1e-12 * np.trace(G) / E)
    L = np.linalg.cholesky(G)
    return L.astype(np.float32)  # [e, zd] lower


def _pack_L(L):
    P = np.empty((128, NTRI * 128), np.float32)
    for bi, (dz, et) in enumerate(TRI):
        P[:, bi * 128 : (bi + 1) * 128] = L[
            et * 128 : (et + 1) * 128, dz * 128 : (dz + 1) * 128
        ]
    return round_f32r(P)


def plan_Mp(mask):
    mask = np.asarray(mask)
    Mv = [int((mask[b] != 0).sum()) for b in range(B)]
    Mp = max(128, int(-(-max(max(Mv), 1) // 128)) * 128)
    return Mv, Mp


def make_in_maps(querys, key_feats, mask, Wq, Wk, Wv, gq, gk, Wo, bo):
    querys = np.asarray(querys, dtype=np.float32)
    key_feats = np.asarray(key_feats, dtype=np.float32)
    mask = np.asarray(mask)
    gq = np.asarray(gq, dtype=np.float32)
    gk = np.asarray(gk, dtype=np.float32)
    Wq = np.asarray(Wq, dtype=np.float32)
    Wk = np.asarray(Wk, dtype=np.float32)

    Mv, Mp = plan_Mp(mask)
    MT = Mp // 128

    gsq_full = gq * np.float32(SCALE)
    Wq_f = Wq * gsq_full[:, None]
    Wk_f = Wk * gk[:, None]
    lq_p = _pack_L(_chol_factor(Wq))
    lk_p = _pack_L(_chol_factor(Wk))

    qT, kfT, mb = [], [], []
    for b in range(B):
        idx = np.nonzero(mask[b])[0]
        kfc = np.zeros((Mp, E), np.float32)
        kfc[: len(idx)] = key_feats[b][idx]
        mbias = np.full((MT, 128), np.float32(NEG), np.float32)
        mbias.reshape(-1)[: len(idx)] = 0.0
        qT.append(round_f32r(querys[b].T))
        kfT.append(round_f32r(kfc.T))
        mb.append(mbias)

    wqT, wkT, wvT, woT = [], [], [], []
    for j in range(4):
        dsl = slice(j * DS, (j + 1) * DS)
        wqT.append(round_f32r(Wq_f[dsl].T))
        wkT.append(round_f32r(Wk_f[dsl].T))
        wvT.append(round_f32r(np.asarray(Wv)[dsl].T))
        woT.append(round_f32r(np.asarray(Wo)[:, dsl].T))

    in_maps = []
    for cid in range(NCORES):
        b, j = cid // 4, cid % 4
        in_maps.append(
            {
                "qT": qT[b],
                "kfT": kfT[b],
                "wqT": wqT[j],
                "wkT": wkT[j],
                "wvT": wvT[j],
                "woT": woT[j],
                "Lq": lq_p,
                "Lk": lk_p,
                "mbias": mb[b],
            }
        )
    return in_maps


def assemble(results, mask, bo):
    mask = np.asarray(mask)
    bo = np.asarray(bo, dtype=np.float32)
    out = np.zeros((B, N, D), dtype=np.float32)
    for cid in range(NCORES):
        b = cid // 4
        out[b] += results[cid]["outT"].T
    out += bo
    for b in range(B):
        if (mask[b] != 0).sum() == 0:
            out[b] = bo  # reference: all-masked row -> attn = 0
    return out


def kernel(querys, key_feats, mask, Wq, Wk, Wv, gq, gk, Wo, bo):
    _, Mp = plan_Mp(mask)
    nc = _get_nc(Mp)
    in_maps = make_in_maps(querys, key_feats, mask, Wq, Wk, Wv, gq, gk, Wo, bo)
    res = run_bass_kernel_spmd(nc, in_maps, list(range(NCORES)))
    return assemble(res.results, mask, bo)
